# revision 54
# baseline (speedup 1.0000x reference)
"""NeuralWDRC Trainium2 kernel: 8-core data-parallel (2 samples/core).

Rewritten from the 100.3us baseline around the TimelineSim cost model:
  - gru is transposed + bf16-cast on the HOST, staged as one resident
    [128, 16000] tile (chunk-interleaved halves) -> no PE transposes, no
    ACT PSUM->SBUF copies in the MLP; MLP chunks software-pipelined one
    chunk ahead (front = yps/a2xt matmuls + ay, back = b2xt + t2 + p2).
  - interp-of-reciprocal: 1/ratio is taken on the [125, 32] ratio grid
    (tiny) and the interp matmul emits E0 = 0.1*interp(rinv) - 0.1
    directly (const term via the ACT-copy bias) -> no [125, 2560]
    reciprocals for the interp path.
  - residual = noisy - enhanced on Pool (device-side, in halves).
  - engine split: DVE env (|x| via STT) / recips / u / gains / scans /
    combine-s1 / reduces / final scales; ACT MLP epilogues + ratio chain
    + E0 copies + u0-s1 (deprioritized to the post-MLP window); Pool
    residual subtract + combine-s0 elementwise; PE matmuls + tail
    partition reduce/broadcast via transpose/ones-matmul (instead of
    the ~2us gpsimd partition ops).
  - queue discipline: audio + xcat pieces + sh3 row-spreads + scan-halo
    shifts split across sync/scalar HWDGE queues so no dependent DMA
    head-blocks the input stream; stores in 8 pieces over 3 queues.
  - weight loads packed into 2 DMAs; audio DMAs lead so DVE part-A
    starts ~4us in; emax reduces fill DVE dependency stalls.
"""

import math
import numpy as np
import ml_dtypes

import concourse.bass as bass
import concourse.bacc as bacc
import concourse.mybir as mybir
import concourse.tile as tile
from concourse.bass_utils import run_bass_kernel_spmd
from concourse import bass_isa

F32 = mybir.dt.float32
BF16 = mybir.dt.bfloat16
AF = mybir.ActivationFunctionType
OP = mybir.AluOpType
NPBF16 = ml_dtypes.bfloat16

NCORES = 8
S = 2                 # samples per core
T = 4000              # MLP timesteps per sample
TB = S * T            # 8000
NSAMP = 320000        # audio samples per batch item
HOP = 80
GRU_H, H1, H2 = 256, 128, 64

P = 125               # audio partitions (125 * 2560 = 320000)
CH = 2560             # chunk cols per partition
W = 48                # halo (warmup) cols (0.9^48 ~ 6.4e-3)
HH = CH // 2          # 1280: column-half
AH = W + HH           # 1344
CHK = 500             # MLP t-chunk
NCHK = TB // CHK      # 16
TTJ = 125             # t subtile
LAM = 2.0 ** -10

# packed bf16 weight layout: name -> (col offset, rows, cols)
BW = {
    "w1t0": (0, 128, 128), "w1t1": (128, 128, 128),
    "a2xt0": (256, 128, 65), "a2xt1": (321, 128, 65),
    "b2xt": (386, 128, 65), "ident": (451, 128, 128),
    "m3d": (579, 36, HOP),
}
BWCOLS = 579 + HOP    # 659
FPCOLS = 4 + 125      # r3|bias1|bias2|spbias + f32 identity

_compiled = {}


def _prep_weights(W1, b1, a1, W2, b2, a2, W3, b3):
    """Host-side weight composition (float64 for accuracy)."""
    W1 = W1.astype(np.float64); W2 = W2.astype(np.float64)
    w3 = W3.astype(np.float64)[2]          # only the ratio output row
    b1 = b1.astype(np.float64); b2 = b2.astype(np.float64)
    b3r = float(np.asarray(b3, np.float64)[2])
    a1 = float(a1); a2 = float(a2)
    c1, d1 = (1 + a1) / 2, (1 - a1) / 2
    c2, d2 = (1 + a2) / 2, (1 - a2) / 2

    A2 = c1 * (W2 @ W1)                    # [64, 256]
    B2 = d1 * W2                           # [64, 128]
    beta2 = b2 + c1 * (W2 @ b1)            # [64]

    a3 = c2 * (A2.T @ w3)                  # [256]
    b3v = c2 * (B2.T @ w3)                 # [128]
    c3v = d2 * w3                          # [64]
    gamma = c2 * float(w3 @ beta2) + b3r

    A2x = np.concatenate([A2, LAM * a3[None, :]], 0)    # [65, 256]
    B2x = np.concatenate([B2, LAM * b3v[None, :]], 0)   # [65, 128]
    beta2x = np.concatenate([beta2, [1.0]])             # [65]
    r3 = np.concatenate([c3v, [1.0 / LAM]])             # [65]
    spb = gamma - 1.0 / LAM                              # scalar

    W1T = W1.T                                           # [256, 128]
    return {
        "w1t0": W1T[:128], "w1t1": W1T[128:],
        "a2xt0": A2x.T[:128], "a2xt1": A2x.T[128:],      # [128, 65]
        "b2xt": B2x.T,                                   # [128, 65]
        "r3": r3, "bias1": b1, "bias2": beta2x, "spb": spb,
    }


def _interp_m3():
    """[3, 80] weights: x_i[80t+k] = sum_j M3[j,k] * x[t-1+j] (clamped)."""
    m = np.zeros((3, HOP), np.float64)
    for k in range(HOP):
        f = (k + 0.5) / HOP - 0.5
        if k < HOP // 2:
            m[0, k] = -f
            m[1, k] = 1.0 + f
        else:
            m[1, k] = 1.0 - f
            m[2, k] = f
    return m


def _build_nc(sim=False):
    nc = bacc.Bacc("TRN2", target_bir_lowering=False, debug=False,
                   num_devices=NCORES)
    xcat = nc.dram_tensor("xcat", [128, NCHK * 2 * CHK], BF16,
                          kind="ExternalInput")
    enh = nc.dram_tensor("enh", [S, NSAMP], BF16, kind="ExternalInput")
    noi = nc.dram_tensor("noi", [S, NSAMP], BF16, kind="ExternalInput")
    wpk = nc.dram_tensor("wpk", [128, BWCOLS], BF16, kind="ExternalInput")
    fpk = nc.dram_tensor("fpk", [128, FPCOLS], F32, kind="ExternalInput")
    out = nc.dram_tensor("out", [S, NSAMP], BF16, kind="ExternalOutput")
    cc_in = nc.dram_tensor("cc_in", [2], F32)
    cc_out = nc.dram_tensor("cc_out", [2 * NCORES], F32, addr_space="Shared")

    ncc = S * T // TTJ  # 64 p2 columns
    rows = ncc // S     # 32 ratio rows per sample
    with tile.TileContext(nc) as tc:
        with (
            tc.tile_pool(name="wpool", bufs=1) as wpool,
            tc.tile_pool(name="mlp", bufs=3) as mlp,
            tc.tile_pool(name="small", bufs=1) as small,
            tc.tile_pool(name="aud", bufs=1) as aud,
            tc.tile_pool(name="gain", bufs=1) as gain,
            tc.tile_pool(name="scr", bufs=2) as scr,
            tc.tile_pool(name="psy", bufs=2, space="PSUM") as psy,
            tc.tile_pool(name="psz", bufs=2, space="PSUM") as psz,
            tc.tile_pool(name="ps1", bufs=1, space="PSUM") as ps1,
            tc.tile_pool(name="ps2", bufs=2, space="PSUM") as ps2,
        ):
            # ---- resident weights: 2 packed DMAs, issued before the ACT
            # table pin so the scalar queue fires them immediately ----
            wsb = wpool.tile([128, BWCOLS], BF16, tag="wsb")
            nc.scalar.dma_start(wsb[:], wpk[:])
            fsb = wpool.tile([128, FPCOLS], F32, tag="fsb")
            nc.scalar.dma_start(fsb[:], fpk[:])
            # pin the exp+ln table once; set 6 covers exp/ln/relu/abs/copy
            nc.scalar.add_instruction(mybir.InstLoadActFuncSet(
                name=nc.get_next_instruction_name(), act_func_set_id=6,
                ins=[], outs=[]))

            def wv(n):
                off, r, c = BW[n]
                return wsb[0:r, off:off + c]
            r3v = fsb[0:65, 0:1]
            bias1 = fsb[0:128, 1:2]
            bias2 = fsb[0:65, 2:3]
            spbias = fsb[0:P, 3:4]
            identf = fsb[0:P, 4:4 + P]
            identb = wv("ident")

            # ---- audio + gru staging on the sync queue only (no dependent
            # DMA ever enters it, so it can never head-block) ----
            xc = wpool.tile([128, NCHK * 2 * CHK], BF16, tag="xc")
            NXP = 8
            XPW = (NCHK * 2 * CHK) // NXP  # 2000 cols per piece
            enh_t0 = aud.tile([P, CH], BF16, tag="enh0")
            res_t0 = aud.tile([P, CH], BF16, tag="res0")
            enh_t1 = aud.tile([P, CH], BF16, tag="enh1")
            res_t1 = aud.tile([P, CH], BF16, tag="res1")
            noi_t0 = aud.tile([P, CH], BF16, tag="noi0")
            noi_t1 = aud.tile([P, CH], BF16, tag="noi1")
            audio_sr = [(enh_t0, res_t0), (enh_t1, res_t1)]
            r2d = lambda ap: ap.rearrange("(p n) -> p n", p=P)
            nc.sync.dma_start(enh_t0[:, 0:HH], r2d(enh[0])[:, 0:HH])
            nc.sync.dma_start(enh_t0[:, HH:CH], r2d(enh[0])[:, HH:CH])
            nc.sync.dma_start(xc[:, 0:XPW], xcat[:, 0:XPW])
            nc.sync.dma_start(noi_t0[:], r2d(noi[0]))
            nc.sync.dma_start(xc[:, XPW:2 * XPW], xcat[:, XPW:2 * XPW])
            nc.sync.dma_start(enh_t1[:], r2d(enh[1]))
            nc.sync.dma_start(xc[:, 2 * XPW:3 * XPW],
                              xcat[:, 2 * XPW:3 * XPW])
            nc.sync.dma_start(noi_t1[:], r2d(noi[1]))
            for i in range(3, NXP):
                nc.sync.dma_start(xc[:, i * XPW:(i + 1) * XPW],
                                  xcat[:, i * XPW:(i + 1) * XPW])
            # residual on-device (model compute stays on the NeuronCore);
            # Pool halves so part-A's ci=1 chain starts as soon as possible
            for s_, (noi_t_, res_t_) in ((0, (noi_t0, res_t0)),
                                         (1, (noi_t1, res_t1))):
                enh_t_ = audio_sr[s_][0]
                for h_ in range(2):
                    hs_ = slice(HH * h_, HH * (h_ + 1))
                    nc.gpsimd.tensor_tensor(res_t_[:, hs_], noi_t_[:, hs_],
                                            enh_t_[:, hs_], op=OP.subtract)

            # scan decay tensors + constants (Pool, after DMA issues)
            d0a = wpool.tile([P, AH], F32, tag="d0a")
            d0b = wpool.tile([P, AH], F32, tag="d0b")
            nc.gpsimd.memset(d0a[:], 0.9)
            nc.gpsimd.memset(d0a[0:1, W:W + 1], 0.0)
            nc.gpsimd.memset(d0b[:], 0.9)
            cb = small.tile([P, 2], F32, tag="cb")
            nc.gpsimd.memset(cb[:, 0:1], 1.0)
            nc.gpsimd.memset(cb[:, 1:2], 1e-8)
            onesf = small.tile([1, 128], F32, tag="onesf")
            nc.gpsimd.memset(onesf[:], 1.0)

            p2m = ps1.tile([P, ncc + 4], F32, tag="p2")
            p2ps = p2m[:, 0:ncc]

            def pe_touch(ap):
                # Absorb one cross-engine dep into a trivial PE matmul so the
                # following self-loading matmuls carry <=1 sync wait.
                if mybir.dt.size(ap.dtype) == 2:
                    a = ap[0:1, 0:2].bitcast(F32)
                else:
                    a = ap[0:1, 0:1].bitcast(F32)
                nc.tensor.matmul(p2m[0:1, ncc + 2:ncc + 3], a, a,
                                 start=True, stop=True)

            vmax = small.tile([P, 8], F32, tag="vmax")
            emax = small.tile([P, 2 * S], F32, tag="emax")
            vout = [None, None]
            sh3 = wpool.tile([36, T + 34], BF16, tag="sh3")
            wq = [nc.sync, nc.scalar]

            # ---- part-A: env -> rec -> u for both samples; fills DVE under
            # the MLP. env = max(-x, x) on DVE (recip(0)=NaN is cleaned by
            # DVE min/max-with-0 downstream); u0 on ACT (deprioritized so it
            # never delays MLP epilogues); u1 on DVE (NaN-safe relu). ----
            us = [[aud.tile([P, CH], BF16, tag=f"u{s}{ci}",
                            name=f"us{s}{ci}") for ci in (0, 1)]
                  for s in range(S)]
            def part_a(s):
                enh_t, res_t = audio_sr[s]
                for h in range(2):
                    with tc.high_priority(offset=-1000000):
                        nc.vector.tensor_reduce(
                            emax[:, 2 * s + h:2 * s + h + 1],
                            enh_t[:, HH * h:HH * (h + 1)], op=OP.max,
                            axis=mybir.AxisListType.X,
                            apply_absolute_value=True)
                for ci, (x, thr) in enumerate(((enh_t, 0.3), (res_t, 0.1))):
                    for h in range(2):
                        hs = slice(HH * h, HH * (h + 1))
                        env = scr.tile([P, HH], F32, tag="env")
                        nc.vector.scalar_tensor_tensor(
                            env[:], in0=x[:, hs], scalar=-1.0,
                            in1=x[:, hs], op0=OP.mult, op1=OP.max)
                        rec = scr.tile([P, HH], F32, tag="rec")
                        nc.vector.reciprocal_approx_fast(out=rec[:],
                                                         in_=env[:])
                        uv = us[s][ci][:, hs]
                        if ci == 0:
                            # u0 not relu'd; the min-trick in the gain stage
                            # (E0 < 0) turns min(E0*u, 0) into E0*relu(u);
                            # s1's copy rides ACT in the post-MLP window
                            if s == 0:
                                nc.vector.tensor_scalar(uv, rec[:], -thr, 1.0,
                                                        op0=OP.mult,
                                                        op1=OP.add)
                            else:
                                with tc.high_priority(offset=-1000000):
                                    nc.scalar.activation(uv, rec[:], AF.Copy,
                                                         bias=1.0, scale=-thr)
                        else:
                            # DVE relu: max(NaN,0)=0 keeps recip(0) safe
                            nc.vector.tensor_scalar(uv, rec[:], -thr, 1.0,
                                                    op0=OP.mult, op1=OP.add)
                            nc.vector.tensor_scalar(uv, uv, 0.0, None,
                                                    op0=OP.max)

            part_a(0)
            part_a(1)
            ratb_t = [None, None]

            def ratio_block(s):
                # softplus on ACT, then 1/ratio on the tiny [125, 32] grid:
                # ratio = clip(sp+1, 1, 20) -> rinv = max(1/(sp+1), 0.05)
                sp = small.tile([P, rows], F32, tag=f"sp{s}")
                nc.scalar.activation(sp[:], p2ps[:, s * rows:(s + 1) * rows],
                                     AF.Exp, bias=spbias)
                nc.scalar.activation(sp[:], sp[:], AF.Ln, bias=cb[:, 0:1])
                rp1 = small.tile([P, rows], F32, tag=f"rp{s}")
                nc.vector.tensor_scalar(rp1[:], sp[:], 1.0, None, op0=OP.add)
                rin = small.tile([P, rows], F32, tag=f"ri{s}")
                nc.vector.reciprocal_approx_fast(out=rin[:], in_=rp1[:])
                ratb = small.tile([P, rows], BF16, tag=f"ratb{s}")
                nc.vector.tensor_scalar(ratb[:], rin[:], 0.05, None,
                                        op0=OP.max)
                ratb_t[s] = ratb

            sm_t = [None, None]

            def sample_block(s):
                b = 32 * s
                ratb = ratb_t[s]
                ratT_ps = ps2.tile([rows, 128], BF16, tag="ri")
                pe_touch(ratb)
                nc.tensor.transpose(ratT_ps[:, 0:P], ratb[:],
                                    identb[:P, :P])
                ratT = small.tile([rows, P], BF16, tag=f"ratT{s}")
                nc.scalar.copy(ratT[:], ratT_ps[:, 0:P])

                # shifted-rinv rows: row b+j, col c = rinv_s[c-1+j] (clamped)
                # all on the Pool SWDGE queue -- they wait on ratT, and no
                # independent DMA queues behind them
                rT = ratT[:]
                r3d = lambda ap: ap.rearrange("p (r q) -> p r q", q=P)
                nc.sync.dma_start(r3d(sh3[b:b + 1, 1:T + 1]), rT)
                nc.scalar.dma_start(sh3[b:b + 1, 0:1], rT[0:1, 0:1])
                nc.scalar.dma_start(r3d(sh3[b + 1:b + 2, 0:T]), rT)
                nc.sync.dma_start(sh3[b + 2:b + 3, 0:124], rT[0:1, 1:P])
                nc.sync.dma_start(
                    r3d(sh3[b + 2:b + 3, 124:124 + 31 * P]), rT[1:rows, :])
                nc.scalar.dma_start(sh3[b + 2:b + 3, T - 1:T],
                                    rT[rows - 1:rows, P - 1:P])

                # ---- interp matmuls emit 0.1*interp(rinv); the -0.1 const
                # rides the ACT copy bias -> E0 [125, 2560] bf16 ----
                pe_touch(sh3[b:b + 1, 0:2])
                E0 = gain.tile([P, CH], BF16, tag="E0")
                ngrp = (rows + 5) // 6
                for g in range(ngrp):
                    taus = list(range(g * 6, min((g + 1) * 6, rows)))
                    rips = ps2.tile([P, 480], F32, tag="ri")
                    for ti, tau in enumerate(taus):
                        lhsT = sh3[b:b + 3, tau:tau + 32 * P:32]  # [3, 125]
                        nc.tensor.matmul(rips[:, ti * HOP:(ti + 1) * HOP],
                                         lhsT, wv("m3d")[b:b + 3, :],
                                         start=True, stop=True)
                    nwid = len(taus) * HOP
                    nc.scalar.activation(E0[:, g * 480:g * 480 + nwid],
                                         rips[:, 0:nwid], AF.Copy, bias=-0.1)
                E1 = gain.tile([P, CH], BF16, tag="E1")
                if s == 0:
                    nc.vector.tensor_scalar(E1[:], E0[:], 0.2, 0.01,
                                            op0=OP.mult, op1=OP.add)
                else:
                    nc.scalar.activation(E1[:], E0[:], AF.Copy,
                                         bias=0.01, scale=0.2)

                # ---- gains w = E*u + kappa, then piecewise scans: s0 in
                # 2 halves, s1 in 4 quarters (shorter critical tail) ----
                NP_ = 2
                PW = CH // NP_          # piece width
                PA = W + PW             # piece + halo
                sm = [None, None]
                for ci, (E, kap) in enumerate(((E0, 0.1), (E1, 0.01))):
                    wf = gain.tile([P, 4 * (W + CH // 4)], BF16,
                                   tag=f"wf{ci}")
                    u = us[s][ci]
                    for q in range(NP_):
                        qs = slice(PW * q, PW * (q + 1))
                        wh = wf[:, PA * q + W:PA * (q + 1)]
                        nc.vector.tensor_tensor(wh, E[:, qs], u[:, qs],
                                                op=OP.mult)
                        if ci == 0:
                            # u unrelu'd; E0<0 makes min(E0*u,0)=E0*relu(u)
                            nc.vector.tensor_scalar(wh, wh, 0.0, kap,
                                                    op0=OP.min, op1=OP.add)
                        else:
                            nc.vector.tensor_scalar(wh, wh, kap, None,
                                                    op0=OP.add)
                        # halo: piece q head <- tail of piece q-1 (same
                        # partition, DVE copy); piece 0 via partition shift
                        if q > 0:
                            nc.vector.tensor_copy(wf[:, PA * q:PA * q + W],
                                                  wf[:, PA * q - W:PA * q])
                    (nc.sync if s == 0 else nc.scalar).dma_start(
                        wf[1:P, 0:W], wf[0:P - 1, NP_ * PA - W:NP_ * PA])
                    nc.vector.memset(wf[0:1, 0:W], 0.0)
                    # stream-start fixup: first real col of partition 0 x10
                    nc.vector.tensor_scalar(wf[0:1, W:W + 1],
                                            wf[0:1, W:W + 1], 10.0, None,
                                            op0=OP.mult)
                    smc = gain.tile([P, 4 * (W + CH // 4)], BF16,
                                    tag=f"sm{ci}")
                    for q in range(NP_):
                        d0q = d0a if q == 0 else d0b
                        nc.vector.tensor_tensor_scan(
                            smc[:, PA * q:PA * (q + 1)], d0q[:, 0:PA],
                            wf[:, PA * q:PA * (q + 1)], 0.0,
                            op0=OP.mult, op1=OP.add)
                    sm[ci] = smc
                sm_t[s] = sm

            def combine_block(s):
                enh_t, res_t = audio_sr[s]
                sm = sm_t[s]
                v = aud.tile([P, CH], BF16, tag=f"v{s}", name=f"v{s}")
                eng = nc.gpsimd if s == 0 else nc.vector
                NP_ = 2
                PW = CH // NP_
                PA = W + PW
                for q in range(NP_):
                    qs = slice(PW * q, PW * (q + 1))
                    ms = slice(PA * q + W, PA * (q + 1))
                    t0 = scr.tile([P, PW], BF16, tag="t0")
                    eng.tensor_tensor(t0[:], enh_t[:, qs], sm[0][:, ms],
                                      op=OP.mult)
                    t1 = scr.tile([P, PW], BF16, tag="t1")
                    eng.tensor_tensor(t1[:], res_t[:, qs],
                                      sm[1][:, ms], op=OP.mult)
                    eng.tensor_tensor(v[:, qs], t0[:], t1[:], op=OP.add)
                    vc = NP_ * s + q
                    nc.vector.tensor_reduce(
                        vmax[:, vc:vc + 1], v[:, qs], op=OP.max,
                        axis=mybir.AxisListType.X, apply_absolute_value=True)
                vout[s] = v

            # ---- MLP over 16 chunks, software-pipelined: chunk c+1's
            # x-dependent matmuls issue before chunk c's ay-dependent tail so
            # PE never drains. xcat cols [c*1000, +500) = gruT[0:128],
            # [+500, +1000) = gruT[128:256]. ----
            ayl = [None] * NCHK
            zpl = [None] * NCHK

            def stage_front(c):
                x0 = xc[:, c * 2 * CHK:c * 2 * CHK + CHK]
                x1 = xc[:, c * 2 * CHK + CHK:(c + 1) * 2 * CHK]
                yps = psy.tile([128, CHK], F32, tag="y")
                pe_touch(x0)
                nc.tensor.matmul(yps[:], wv("w1t0"), x0,
                                 start=True, stop=False)
                nc.tensor.matmul(yps[:], wv("w1t1"), x1,
                                 start=False, stop=True)
                zps = psz.tile([65, CHK], F32, tag="z")
                nc.tensor.matmul(zps[:], wv("a2xt0"), x0,
                                 start=True, stop=False)
                nc.tensor.matmul(zps[:], wv("a2xt1"), x1,
                                 start=False, stop=False)
                ay = mlp.tile([128, CHK], BF16, tag="ay")
                nc.scalar.activation(ay[:], yps[:], AF.Abs, bias=bias1)
                ayl[c], zpl[c] = ay, zps

            def stage_back(c):
                ay, zps = ayl[c], zpl[c]
                pe_touch(ay)
                nc.tensor.matmul(zps[:], wv("b2xt"), ay[:],
                                 start=False, stop=True)
                # t2 = |z + beta2x| stays f32 (the 1/LAM row would lose the
                # lambda-encoded payload in bf16)
                t2 = mlp.tile([65, CHK], F32, tag="t2")
                nc.scalar.activation(t2[:], zps[:], AF.Abs, bias=bias2)
                for j in range(CHK // TTJ):
                    cc = c * (CHK // TTJ) + j
                    nc.tensor.matmul(p2ps[:, cc:cc + 1],
                                     t2[:, j * TTJ:(j + 1) * TTJ], r3v,
                                     start=True, stop=True)

            stage_front(0)
            for c in range(NCHK):
                if c + 1 < NCHK:
                    stage_front(c + 1)
                stage_back(c)
                if c == NCHK // 2 - 1:
                    with tc.high_priority():
                        ratio_block(0)
                elif c == NCHK // 2 + 1:
                    with tc.high_priority():
                        sample_block(0)
                elif c == NCHK - 1:
                    with tc.high_priority():
                        ratio_block(1)
                        sample_block(1)
                    combine_block(0)
                    with tc.high_priority(offset=100):
                        combine_block(1)

            # ---- global normalization; partition reduce via PE transpose
            # (gpsimd partition_all_reduce is ~2us, this is ~0.4us) ----
            gms = small.tile([P, 2], F32, tag="gms")
            nc.vector.tensor_reduce(gms[:, 0:1], vmax[:, 0:4], op=OP.max,
                                    axis=mybir.AxisListType.X)
            nc.vector.tensor_reduce(gms[:, 1:2], emax[:], op=OP.max,
                                    axis=mybir.AxisListType.X)
            tps = ps2.tile([2, 128], F32, tag="ri")
            pe_touch(gms)
            nc.tensor.transpose(tps[0:2, 0:P], gms[:], identf)
            gg = small.tile([2, 1], F32, tag="gg")
            nc.vector.tensor_reduce(gg[:], tps[0:2, 0:P], op=OP.max,
                                    axis=mybir.AxisListType.X)
            ccsb = small.tile([1, 2 * NCORES], F32, tag="ccsb")
            nc.gpsimd.memset(ccsb[:], 0.0)
            if sim:
                nc.sync.dma_start(ccsb[0:1, 0:2], gg[0:2, 0:1])
            else:
                with tc.tile_critical():
                    cc_sem = nc.alloc_semaphore("ccs")
                    nc.gpsimd.dma_start(cc_in[:], gg[0:2, 0:1]).then_inc(
                        cc_sem, 16)
                    nc.gpsimd.collective_compute(
                        "AllGather", OP.bypass,
                        replica_groups=[list(range(NCORES))],
                        ins=[cc_in[:]], outs=[cc_out[:]],
                    )._wait_ge(cc_sem, 16).then_inc(cc_sem, 1)
                    nc.gpsimd.dma_start(ccsb[:], cc_out[None, :])._wait_ge(
                        cc_sem, 17).then_inc(cc_sem, 16)
                    nc.gpsimd.engine_nop()._wait_ge(cc_sem, 33)

            sg = small.tile([1, 4], F32, tag="sg")
            nc.vector.tensor_reduce(sg[:, 0:1], ccsb[:, 0:2 * NCORES:2],
                                    op=OP.max, axis=mybir.AxisListType.X)
            nc.vector.tensor_reduce(sg[:, 1:2], ccsb[:, 1:2 * NCORES:2],
                                    op=OP.max, axis=mybir.AxisListType.X)
            # sigma = emax / (vmax + 1e-8)
            nc.vector.tensor_scalar(sg[:, 2:3], sg[:, 0:1], 1e-8, None,
                                    op0=OP.add)
            nc.vector.reciprocal_approx_fast(out=sg[:, 0:1], in_=sg[:, 2:3])
            nc.vector.tensor_tensor(sg[:, 3:4], sg[:, 0:1], sg[:, 1:2],
                                    op=OP.mult)
            # broadcast sigma to all partitions via PE (ones-row matmul)
            sgp = ps2.tile([P, 8], F32, tag="ri")
            pe_touch(sg)
            nc.tensor.matmul(sgp[:, 0:1], onesf[0:1, 0:P], sg[0:1, 3:4],
                             start=True, stop=True)
            sgb = small.tile([P, 1], F32, tag="sgb")
            nc.vector.tensor_copy(sgb[:], sgp[:, 0:1])

            oq = [nc.sync, nc.scalar, nc.sync, nc.scalar, nc.sync]
            pieces = [(1, HH + HH // 2, CH), (1, HH, HH + HH // 2),
                      (0, 0, HH), (0, HH, CH), (1, 0, HH)]
            for i, (s, lo, hi) in enumerate(pieces):
                oview = out[s].rearrange("(p n) -> p n", p=P)
                qs = slice(lo, hi)
                nc.vector.tensor_scalar(vout[s][:, qs], vout[s][:, qs],
                                        sgb[:, 0:1], None, op0=OP.mult)
                oq[i].dma_start(oview[:, qs], vout[s][:, qs])
    nc.finalize()
    return nc


def _host_prep(inputs):
    gru = np.ascontiguousarray(np.asarray(inputs["gru_output"], np.float32))
    enh = np.ascontiguousarray(np.asarray(inputs["enhanced"], np.float32))
    noisy = np.ascontiguousarray(np.asarray(inputs["noisy"], np.float32))
    B = gru.shape[0]
    wts = _prep_weights(inputs["W1"], inputs["b1"], inputs["a1"],
                        inputs["W2"], inputs["b2"], inputs["a2"],
                        inputs["W3"], inputs["b3"])
    # bf16 weight pack
    wpk = np.zeros((128, BWCOLS), NPBF16)
    for n in ("w1t0", "w1t1", "a2xt0", "a2xt1", "b2xt"):
        off, r, c = BW[n]
        wpk[0:r, off:off + c] = wts[n].astype(NPBF16)
    off, r, c = BW["ident"]
    wpk[0:r, off:off + c] = np.eye(128).astype(NPBF16)
    m3 = _interp_m3()
    off, r, c = BW["m3d"]
    m3d = np.zeros((36, HOP))
    m3d[0:3] = 0.1 * m3
    m3d[32:35] = 0.1 * m3
    wpk[0:r, off:off + c] = m3d.astype(NPBF16)
    # f32 pack: r3 | bias1 | bias2 | spbias | eye(125)
    fpk = np.zeros((128, FPCOLS), np.float32)
    fpk[0:65, 0] = wts["r3"]
    fpk[0:128, 1] = wts["bias1"]
    fpk[0:65, 2] = wts["bias2"]
    fpk[0:P, 3] = wts["spb"]
    fpk[0:P, 4:4 + P] = np.eye(P, dtype=np.float32)

    noib = noisy.astype(NPBF16)
    enhb = enh.astype(NPBF16)

    per = B // NCORES
    in_maps = []
    for cr in range(NCORES):
        g = gru[cr * per:(cr + 1) * per].reshape(TB, GRU_H)
        gT = np.ascontiguousarray(g.T.astype(NPBF16))      # [256, 8000]
        xcat = np.empty((128, NCHK * 2 * CHK), NPBF16)
        for ch in range(NCHK):
            xcat[:, ch * 2 * CHK:ch * 2 * CHK + CHK] = \
                gT[0:128, ch * CHK:(ch + 1) * CHK]
            xcat[:, ch * 2 * CHK + CHK:(ch + 1) * 2 * CHK] = \
                gT[128:256, ch * CHK:(ch + 1) * CHK]
        m = {
            "xcat": xcat,
            "enh": np.ascontiguousarray(enhb[cr * per:(cr + 1) * per]),
            "noi": np.ascontiguousarray(noib[cr * per:(cr + 1) * per]),
            "wpk": wpk, "fpk": fpk,
        }
        in_maps.append(m)
    return in_maps


def kernel(trace=False, **inputs):
    in_maps = _host_prep(inputs)
    if "nc" not in _compiled:
        _compiled["nc"] = _build_nc()
    nc = _compiled["nc"]
    res = run_bass_kernel_spmd(nc, in_maps, list(range(NCORES)), trace=trace)
    outs = [np.asarray(res.results[c]["out"]).astype(np.float32)
            for c in range(NCORES)]
    full = np.concatenate(outs, axis=0)
    if trace:
        return full, res
    return full


if __name__ == "__main__":
    pass


# revision 55
# speedup vs baseline: 1.0023x; 1.0023x over previous
"""NeuralWDRC Trainium2 kernel: 8-core data-parallel (2 samples/core).

Rewritten from the 100.3us baseline around the TimelineSim cost model:
  - gru is transposed + bf16-cast on the HOST, staged as one resident
    [128, 16000] tile (chunk-interleaved halves) -> no PE transposes, no
    ACT PSUM->SBUF copies in the MLP; MLP chunks software-pipelined one
    chunk ahead (front = yps/a2xt matmuls + ay, back = b2xt + t2 + p2).
  - interp-of-reciprocal: 1/ratio is taken on the [125, 32] ratio grid
    (tiny) and the interp matmul emits E0 = 0.1*interp(rinv) - 0.1
    directly (const term via the ACT-copy bias) -> no [125, 2560]
    reciprocals for the interp path.
  - residual = noisy - enhanced on Pool (device-side, in halves).
  - engine split: DVE env (|x| via STT) / recips / u / gains / scans /
    combine-s1 / reduces / final scales; ACT MLP epilogues + ratio chain
    + E0 copies + u0-s1 (deprioritized to the post-MLP window); Pool
    residual subtract + combine-s0 elementwise; PE matmuls + tail
    partition reduce/broadcast via transpose/ones-matmul (instead of
    the ~2us gpsimd partition ops).
  - queue discipline: audio + xcat pieces + sh3 row-spreads + scan-halo
    shifts split across sync/scalar HWDGE queues so no dependent DMA
    head-blocks the input stream; stores in 8 pieces over 3 queues.
  - weight loads packed into 2 DMAs; audio DMAs lead so DVE part-A
    starts ~4us in; emax reduces fill DVE dependency stalls.
"""

import math
import numpy as np
import ml_dtypes

import concourse.bass as bass
import concourse.bacc as bacc
import concourse.mybir as mybir
import concourse.tile as tile
from concourse.bass_utils import run_bass_kernel_spmd
from concourse import bass_isa

F32 = mybir.dt.float32
BF16 = mybir.dt.bfloat16
AF = mybir.ActivationFunctionType
OP = mybir.AluOpType
NPBF16 = ml_dtypes.bfloat16

NCORES = 8
S = 2                 # samples per core
T = 4000              # MLP timesteps per sample
TB = S * T            # 8000
NSAMP = 320000        # audio samples per batch item
HOP = 80
GRU_H, H1, H2 = 256, 128, 64

P = 125               # audio partitions (125 * 2560 = 320000)
CH = 2560             # chunk cols per partition
W = 48                # halo (warmup) cols (0.9^48 ~ 6.4e-3)
HH = CH // 2          # 1280: column-half
AH = W + HH           # 1344
CHK = 500             # MLP t-chunk
NCHK = TB // CHK      # 16
TTJ = 125             # t subtile
LAM = 2.0 ** -10

# packed bf16 weight layout: name -> (col offset, rows, cols)
BW = {
    "w1t0": (0, 128, 128), "w1t1": (128, 128, 128),
    "a2xt0": (256, 128, 65), "a2xt1": (321, 128, 65),
    "b2xt": (386, 128, 65), "ident": (451, 128, 128),
    "m3d": (579, 36, HOP),
}
BWCOLS = 579 + HOP    # 659
FPCOLS = 4 + 125      # r3|bias1|bias2|spbias + f32 identity

_compiled = {}


def _prep_weights(W1, b1, a1, W2, b2, a2, W3, b3):
    """Host-side weight composition (float64 for accuracy)."""
    W1 = W1.astype(np.float64); W2 = W2.astype(np.float64)
    w3 = W3.astype(np.float64)[2]          # only the ratio output row
    b1 = b1.astype(np.float64); b2 = b2.astype(np.float64)
    b3r = float(np.asarray(b3, np.float64)[2])
    a1 = float(a1); a2 = float(a2)
    c1, d1 = (1 + a1) / 2, (1 - a1) / 2
    c2, d2 = (1 + a2) / 2, (1 - a2) / 2

    A2 = c1 * (W2 @ W1)                    # [64, 256]
    B2 = d1 * W2                           # [64, 128]
    beta2 = b2 + c1 * (W2 @ b1)            # [64]

    a3 = c2 * (A2.T @ w3)                  # [256]
    b3v = c2 * (B2.T @ w3)                 # [128]
    c3v = d2 * w3                          # [64]
    gamma = c2 * float(w3 @ beta2) + b3r

    A2x = np.concatenate([A2, LAM * a3[None, :]], 0)    # [65, 256]
    B2x = np.concatenate([B2, LAM * b3v[None, :]], 0)   # [65, 128]
    beta2x = np.concatenate([beta2, [1.0]])             # [65]
    r3 = np.concatenate([c3v, [1.0 / LAM]])             # [65]
    spb = gamma - 1.0 / LAM                              # scalar

    W1T = W1.T                                           # [256, 128]
    return {
        "w1t0": W1T[:128], "w1t1": W1T[128:],
        "a2xt0": A2x.T[:128], "a2xt1": A2x.T[128:],      # [128, 65]
        "b2xt": B2x.T,                                   # [128, 65]
        "r3": r3, "bias1": b1, "bias2": beta2x, "spb": spb,
    }


def _interp_m3():
    """[3, 80] weights: x_i[80t+k] = sum_j M3[j,k] * x[t-1+j] (clamped)."""
    m = np.zeros((3, HOP), np.float64)
    for k in range(HOP):
        f = (k + 0.5) / HOP - 0.5
        if k < HOP // 2:
            m[0, k] = -f
            m[1, k] = 1.0 + f
        else:
            m[1, k] = 1.0 - f
            m[2, k] = f
    return m


def _build_nc(sim=False):
    nc = bacc.Bacc("TRN2", target_bir_lowering=False, debug=False,
                   num_devices=NCORES)
    xcat = nc.dram_tensor("xcat", [128, NCHK * 2 * CHK], BF16,
                          kind="ExternalInput")
    enh = nc.dram_tensor("enh", [S, NSAMP], BF16, kind="ExternalInput")
    noi = nc.dram_tensor("noi", [S, NSAMP], BF16, kind="ExternalInput")
    wpk = nc.dram_tensor("wpk", [128, BWCOLS], BF16, kind="ExternalInput")
    fpk = nc.dram_tensor("fpk", [128, FPCOLS], F32, kind="ExternalInput")
    out = nc.dram_tensor("out", [S, NSAMP], BF16, kind="ExternalOutput")
    cc_in = nc.dram_tensor("cc_in", [2], F32)
    cc_out = nc.dram_tensor("cc_out", [2 * NCORES], F32, addr_space="Shared")

    ncc = S * T // TTJ  # 64 p2 columns
    rows = ncc // S     # 32 ratio rows per sample
    with tile.TileContext(nc) as tc:
        with (
            tc.tile_pool(name="wpool", bufs=1) as wpool,
            tc.tile_pool(name="mlp", bufs=3) as mlp,
            tc.tile_pool(name="small", bufs=1) as small,
            tc.tile_pool(name="aud", bufs=1) as aud,
            tc.tile_pool(name="gain", bufs=1) as gain,
            tc.tile_pool(name="scr", bufs=2) as scr,
            tc.tile_pool(name="psy", bufs=2, space="PSUM") as psy,
            tc.tile_pool(name="psz", bufs=2, space="PSUM") as psz,
            tc.tile_pool(name="ps1", bufs=1, space="PSUM") as ps1,
            tc.tile_pool(name="ps2", bufs=2, space="PSUM") as ps2,
        ):
            # ---- resident weights: 2 packed DMAs, issued before the ACT
            # table pin so the scalar queue fires them immediately ----
            wsb = wpool.tile([128, BWCOLS], BF16, tag="wsb")
            nc.scalar.dma_start(wsb[:], wpk[:])
            fsb = wpool.tile([128, FPCOLS], F32, tag="fsb")
            nc.scalar.dma_start(fsb[:], fpk[:])
            # pin the exp+ln table once; set 6 covers exp/ln/relu/abs/copy
            nc.scalar.add_instruction(mybir.InstLoadActFuncSet(
                name=nc.get_next_instruction_name(), act_func_set_id=6,
                ins=[], outs=[]))

            def wv(n):
                off, r, c = BW[n]
                return wsb[0:r, off:off + c]
            r3v = fsb[0:65, 0:1]
            bias1 = fsb[0:128, 1:2]
            bias2 = fsb[0:65, 2:3]
            spbias = fsb[0:P, 3:4]
            identf = fsb[0:P, 4:4 + P]
            identb = wv("ident")

            # ---- audio + gru staging on the sync queue only (no dependent
            # DMA ever enters it, so it can never head-block) ----
            xc = wpool.tile([128, NCHK * 2 * CHK], BF16, tag="xc")
            NXP = 8
            XPW = (NCHK * 2 * CHK) // NXP  # 2000 cols per piece
            enh_t0 = aud.tile([P, CH], BF16, tag="enh0")
            res_t0 = aud.tile([P, CH], BF16, tag="res0")
            enh_t1 = aud.tile([P, CH], BF16, tag="enh1")
            res_t1 = aud.tile([P, CH], BF16, tag="res1")
            noi_t0 = aud.tile([P, CH], BF16, tag="noi0")
            noi_t1 = aud.tile([P, CH], BF16, tag="noi1")
            audio_sr = [(enh_t0, res_t0), (enh_t1, res_t1)]
            r2d = lambda ap: ap.rearrange("(p n) -> p n", p=P)
            nc.sync.dma_start(enh_t0[:, 0:HH], r2d(enh[0])[:, 0:HH])
            nc.sync.dma_start(enh_t0[:, HH:CH], r2d(enh[0])[:, HH:CH])
            nc.sync.dma_start(xc[:, 0:XPW], xcat[:, 0:XPW])
            nc.sync.dma_start(noi_t0[:], r2d(noi[0]))
            nc.sync.dma_start(xc[:, XPW:2 * XPW], xcat[:, XPW:2 * XPW])
            nc.sync.dma_start(enh_t1[:], r2d(enh[1]))
            nc.sync.dma_start(xc[:, 2 * XPW:3 * XPW],
                              xcat[:, 2 * XPW:3 * XPW])
            nc.sync.dma_start(noi_t1[:], r2d(noi[1]))
            for i in range(3, NXP):
                nc.sync.dma_start(xc[:, i * XPW:(i + 1) * XPW],
                                  xcat[:, i * XPW:(i + 1) * XPW])
            # residual on-device (model compute stays on the NeuronCore);
            # Pool halves so part-A's ci=1 chain starts as soon as possible
            for s_, (noi_t_, res_t_) in ((0, (noi_t0, res_t0)),
                                         (1, (noi_t1, res_t1))):
                enh_t_ = audio_sr[s_][0]
                for h_ in range(2):
                    hs_ = slice(HH * h_, HH * (h_ + 1))
                    nc.gpsimd.tensor_tensor(res_t_[:, hs_], noi_t_[:, hs_],
                                            enh_t_[:, hs_], op=OP.subtract)

            # scan decay tensors + constants (Pool, after DMA issues)
            d0a = wpool.tile([P, AH], F32, tag="d0a")
            d0b = wpool.tile([P, AH], F32, tag="d0b")
            nc.gpsimd.memset(d0a[:], 0.9)
            nc.gpsimd.memset(d0a[0:1, W:W + 1], 0.0)
            nc.gpsimd.memset(d0b[:], 0.9)
            cb = small.tile([P, 2], F32, tag="cb")
            nc.gpsimd.memset(cb[:, 0:1], 1.0)
            nc.gpsimd.memset(cb[:, 1:2], 1e-8)
            onesf = small.tile([1, 128], F32, tag="onesf")
            nc.gpsimd.memset(onesf[:], 1.0)

            p2m = ps1.tile([P, ncc + 4], F32, tag="p2")
            p2ps = p2m[:, 0:ncc]

            def pe_touch(ap):
                # Absorb one cross-engine dep into a trivial PE matmul so the
                # following self-loading matmuls carry <=1 sync wait.
                if mybir.dt.size(ap.dtype) == 2:
                    a = ap[0:1, 0:2].bitcast(F32)
                else:
                    a = ap[0:1, 0:1].bitcast(F32)
                nc.tensor.matmul(p2m[0:1, ncc + 2:ncc + 3], a, a,
                                 start=True, stop=True)

            vmax = small.tile([P, 8], F32, tag="vmax")
            emax = small.tile([P, 2 * S], F32, tag="emax")
            vout = [None, None]
            sh3 = wpool.tile([36, T + 34], BF16, tag="sh3")
            wq = [nc.sync, nc.scalar]

            # ---- part-A: env -> rec -> u for both samples; fills DVE under
            # the MLP. env = max(-x, x) on DVE (recip(0)=NaN is cleaned by
            # DVE min/max-with-0 downstream); u0 on ACT (deprioritized so it
            # never delays MLP epilogues); u1 on DVE (NaN-safe relu). ----
            us = [[aud.tile([P, CH], BF16, tag=f"u{s}{ci}",
                            name=f"us{s}{ci}") for ci in (0, 1)]
                  for s in range(S)]
            def part_a(s):
                enh_t, res_t = audio_sr[s]
                for h in range(2):
                    with tc.high_priority(offset=-1000000):
                        nc.vector.tensor_reduce(
                            emax[:, 2 * s + h:2 * s + h + 1],
                            enh_t[:, HH * h:HH * (h + 1)], op=OP.max,
                            axis=mybir.AxisListType.X,
                            apply_absolute_value=True)
                for ci, (x, thr) in enumerate(((enh_t, 0.3), (res_t, 0.1))):
                    for h in range(2):
                        hs = slice(HH * h, HH * (h + 1))
                        env = scr.tile([P, HH], F32, tag="env")
                        nc.vector.scalar_tensor_tensor(
                            env[:], in0=x[:, hs], scalar=-1.0,
                            in1=x[:, hs], op0=OP.mult, op1=OP.max)
                        rec = scr.tile([P, HH], F32, tag="rec")
                        nc.vector.reciprocal_approx_fast(out=rec[:],
                                                         in_=env[:])
                        uv = us[s][ci][:, hs]
                        if ci == 0:
                            # u0 not relu'd; the min-trick in the gain stage
                            # (E0 < 0) turns min(E0*u, 0) into E0*relu(u);
                            # s1's copy rides ACT in the post-MLP window
                            if s == 0:
                                nc.vector.tensor_scalar(uv, rec[:], -thr, 1.0,
                                                        op0=OP.mult,
                                                        op1=OP.add)
                            else:
                                with tc.high_priority(offset=-1000000):
                                    nc.scalar.activation(uv, rec[:], AF.Copy,
                                                         bias=1.0, scale=-thr)
                        else:
                            # DVE relu: max(NaN,0)=0 keeps recip(0) safe
                            nc.vector.tensor_scalar(uv, rec[:], -thr, 1.0,
                                                    op0=OP.mult, op1=OP.add)
                            nc.vector.tensor_scalar(uv, uv, 0.0, None,
                                                    op0=OP.max)

            part_a(0)
            part_a(1)
            ratb_t = [None, None]

            def ratio_block(s):
                # softplus on ACT, then 1/ratio on the tiny [125, 32] grid:
                # ratio = clip(sp+1, 1, 20) -> rinv = max(1/(sp+1), 0.05)
                sp = small.tile([P, rows], F32, tag=f"sp{s}")
                nc.scalar.activation(sp[:], p2ps[:, s * rows:(s + 1) * rows],
                                     AF.Exp, bias=spbias)
                nc.scalar.activation(sp[:], sp[:], AF.Ln, bias=cb[:, 0:1])
                rp1 = small.tile([P, rows], F32, tag=f"rp{s}")
                nc.vector.tensor_scalar(rp1[:], sp[:], 1.0, None, op0=OP.add)
                rin = small.tile([P, rows], F32, tag=f"ri{s}")
                nc.vector.reciprocal_approx_fast(out=rin[:], in_=rp1[:])
                ratb = small.tile([P, rows], BF16, tag=f"ratb{s}")
                nc.vector.tensor_scalar(ratb[:], rin[:], 0.05, None,
                                        op0=OP.max)
                ratb_t[s] = ratb

            sm_t = [None, None]

            def sample_block(s):
                b = 32 * s
                ratb = ratb_t[s]
                ratT_ps = ps2.tile([rows, 128], BF16, tag="ri")
                pe_touch(ratb)
                nc.tensor.transpose(ratT_ps[:, 0:P], ratb[:],
                                    identb[:P, :P])
                ratT = small.tile([rows, P], BF16, tag=f"ratT{s}")
                nc.scalar.copy(ratT[:], ratT_ps[:, 0:P])

                # shifted-rinv rows: row b+j, col c = rinv_s[c-1+j] (clamped)
                # all on the Pool SWDGE queue -- they wait on ratT, and no
                # independent DMA queues behind them
                rT = ratT[:]
                r3d = lambda ap: ap.rearrange("p (r q) -> p r q", q=P)
                nc.sync.dma_start(r3d(sh3[b:b + 1, 1:T + 1]), rT)
                nc.scalar.dma_start(sh3[b:b + 1, 0:1], rT[0:1, 0:1])
                nc.scalar.dma_start(r3d(sh3[b + 1:b + 2, 0:T]), rT)
                nc.sync.dma_start(sh3[b + 2:b + 3, 0:124], rT[0:1, 1:P])
                nc.sync.dma_start(
                    r3d(sh3[b + 2:b + 3, 124:124 + 31 * P]), rT[1:rows, :])
                nc.scalar.dma_start(sh3[b + 2:b + 3, T - 1:T],
                                    rT[rows - 1:rows, P - 1:P])

                # ---- interp matmuls emit 0.1*interp(rinv); the -0.1 const
                # rides the ACT copy bias -> E0 [125, 2560] bf16 ----
                pe_touch(sh3[b:b + 1, 0:2])
                E0 = gain.tile([P, CH], BF16, tag="E0")
                ngrp = (rows + 5) // 6
                for g in range(ngrp):
                    taus = list(range(g * 6, min((g + 1) * 6, rows)))
                    rips = ps2.tile([P, 480], F32, tag="ri")
                    for ti, tau in enumerate(taus):
                        lhsT = sh3[b:b + 3, tau:tau + 32 * P:32]  # [3, 125]
                        nc.tensor.matmul(rips[:, ti * HOP:(ti + 1) * HOP],
                                         lhsT, wv("m3d")[b:b + 3, :],
                                         start=True, stop=True)
                    nwid = len(taus) * HOP
                    nc.scalar.activation(E0[:, g * 480:g * 480 + nwid],
                                         rips[:, 0:nwid], AF.Copy, bias=-0.1)
                E1 = gain.tile([P, CH], BF16, tag="E1")
                if s == 0:
                    nc.vector.tensor_scalar(E1[:], E0[:], 0.2, 0.01,
                                            op0=OP.mult, op1=OP.add)
                else:
                    nc.scalar.activation(E1[:], E0[:], AF.Copy,
                                         bias=0.01, scale=0.2)

                # ---- gains w = E*u + kappa, then piecewise scans: s0 in
                # 2 halves, s1 in 4 quarters (shorter critical tail) ----
                NP_ = 2
                PW = CH // NP_          # piece width
                PA = W + PW             # piece + halo
                sm = [None, None]
                for ci, (E, kap) in enumerate(((E0, 0.1), (E1, 0.01))):
                    wf = gain.tile([P, 4 * (W + CH // 4)], BF16,
                                   tag=f"wf{ci}")
                    u = us[s][ci]
                    for q in range(NP_):
                        qs = slice(PW * q, PW * (q + 1))
                        wh = wf[:, PA * q + W:PA * (q + 1)]
                        nc.vector.tensor_tensor(wh, E[:, qs], u[:, qs],
                                                op=OP.mult)
                        if ci == 0:
                            # u unrelu'd; E0<0 makes min(E0*u,0)=E0*relu(u)
                            nc.vector.tensor_scalar(wh, wh, 0.0, kap,
                                                    op0=OP.min, op1=OP.add)
                        else:
                            nc.vector.tensor_scalar(wh, wh, kap, None,
                                                    op0=OP.add)
                        # halo: piece q head <- tail of piece q-1 (same
                        # partition, DVE copy); piece 0 via partition shift
                        if q > 0:
                            nc.vector.tensor_copy(wf[:, PA * q:PA * q + W],
                                                  wf[:, PA * q - W:PA * q])
                    (nc.sync if s == 0 else nc.scalar).dma_start(
                        wf[1:P, 0:W], wf[0:P - 1, NP_ * PA - W:NP_ * PA])
                    nc.vector.memset(wf[0:1, 0:W], 0.0)
                    # stream-start fixup: first real col of partition 0 x10
                    nc.vector.tensor_scalar(wf[0:1, W:W + 1],
                                            wf[0:1, W:W + 1], 10.0, None,
                                            op0=OP.mult)
                    smc = gain.tile([P, 4 * (W + CH // 4)], BF16,
                                    tag=f"sm{ci}")
                    for q in range(NP_):
                        d0q = d0a if q == 0 else d0b
                        nc.vector.tensor_tensor_scan(
                            smc[:, PA * q:PA * (q + 1)], d0q[:, 0:PA],
                            wf[:, PA * q:PA * (q + 1)], 0.0,
                            op0=OP.mult, op1=OP.add)
                    sm[ci] = smc
                sm_t[s] = sm

            def combine_block(s):
                enh_t, res_t = audio_sr[s]
                sm = sm_t[s]
                v = aud.tile([P, CH], BF16, tag=f"v{s}", name=f"v{s}")
                eng = nc.gpsimd if s == 0 else nc.vector
                NP_ = 2
                PW = CH // NP_
                PA = W + PW
                for q in range(NP_):
                    qs = slice(PW * q, PW * (q + 1))
                    ms = slice(PA * q + W, PA * (q + 1))
                    t0 = scr.tile([P, PW], BF16, tag="t0")
                    eng.tensor_tensor(t0[:], enh_t[:, qs], sm[0][:, ms],
                                      op=OP.mult)
                    t1 = scr.tile([P, PW], BF16, tag="t1")
                    eng.tensor_tensor(t1[:], res_t[:, qs],
                                      sm[1][:, ms], op=OP.mult)
                    eng.tensor_tensor(v[:, qs], t0[:], t1[:], op=OP.add)
                    vc = NP_ * s + q
                    nc.vector.tensor_reduce(
                        vmax[:, vc:vc + 1], v[:, qs], op=OP.max,
                        axis=mybir.AxisListType.X, apply_absolute_value=True)
                vout[s] = v

            # ---- MLP over 16 chunks, software-pipelined: chunk c+1's
            # x-dependent matmuls issue before chunk c's ay-dependent tail so
            # PE never drains. xcat cols [c*1000, +500) = gruT[0:128],
            # [+500, +1000) = gruT[128:256]. ----
            ayl = [None] * NCHK
            zpl = [None] * NCHK

            def stage_front(c):
                x0 = xc[:, c * 2 * CHK:c * 2 * CHK + CHK]
                x1 = xc[:, c * 2 * CHK + CHK:(c + 1) * 2 * CHK]
                yps = psy.tile([128, CHK], F32, tag="y")
                pe_touch(x0)
                nc.tensor.matmul(yps[:], wv("w1t0"), x0,
                                 start=True, stop=False)
                nc.tensor.matmul(yps[:], wv("w1t1"), x1,
                                 start=False, stop=True)
                zps = psz.tile([65, CHK], F32, tag="z")
                nc.tensor.matmul(zps[:], wv("a2xt0"), x0,
                                 start=True, stop=False)
                nc.tensor.matmul(zps[:], wv("a2xt1"), x1,
                                 start=False, stop=False)
                ay = mlp.tile([128, CHK], BF16, tag="ay")
                nc.scalar.activation(ay[:], yps[:], AF.Abs, bias=bias1)
                ayl[c], zpl[c] = ay, zps

            def stage_back(c):
                ay, zps = ayl[c], zpl[c]
                pe_touch(ay)
                nc.tensor.matmul(zps[:], wv("b2xt"), ay[:],
                                 start=False, stop=True)
                # t2 = |z + beta2x| stays f32 (the 1/LAM row would lose the
                # lambda-encoded payload in bf16)
                t2 = mlp.tile([65, CHK], F32, tag="t2")
                nc.scalar.activation(t2[:], zps[:], AF.Abs, bias=bias2)
                for j in range(CHK // TTJ):
                    cc = c * (CHK // TTJ) + j
                    nc.tensor.matmul(p2ps[:, cc:cc + 1],
                                     t2[:, j * TTJ:(j + 1) * TTJ], r3v,
                                     start=True, stop=True)

            stage_front(0)
            for c in range(NCHK):
                if c + 1 < NCHK:
                    stage_front(c + 1)
                stage_back(c)
                if c == NCHK // 2 - 1:
                    with tc.high_priority():
                        ratio_block(0)
                elif c == NCHK // 2 + 1:
                    with tc.high_priority():
                        sample_block(0)
                elif c == NCHK - 1:
                    with tc.high_priority():
                        ratio_block(1)
                        sample_block(1)
                    combine_block(0)
                    with tc.high_priority(offset=100):
                        combine_block(1)

            # ---- global normalization; partition reduce via PE transpose
            # (gpsimd partition_all_reduce is ~2us, this is ~0.4us) ----
            gms = small.tile([P, 2], F32, tag="gms")
            nc.vector.tensor_reduce(gms[:, 0:1], vmax[:, 0:4], op=OP.max,
                                    axis=mybir.AxisListType.X)
            nc.vector.tensor_reduce(gms[:, 1:2], emax[:], op=OP.max,
                                    axis=mybir.AxisListType.X)
            tps = ps2.tile([2, 128], F32, tag="ri")
            pe_touch(gms)
            nc.tensor.transpose(tps[0:2, 0:P], gms[:], identf)
            gg = small.tile([2, 1], F32, tag="gg")
            nc.vector.tensor_reduce(gg[:], tps[0:2, 0:P], op=OP.max,
                                    axis=mybir.AxisListType.X)
            ccsb = small.tile([1, 2 * NCORES], F32, tag="ccsb")
            nc.gpsimd.memset(ccsb[:], 0.0)
            if sim:
                nc.sync.dma_start(ccsb[0:1, 0:2], gg[0:2, 0:1])
            else:
                with tc.tile_critical():
                    cc_sem = nc.alloc_semaphore("ccs")
                    nc.gpsimd.dma_start(cc_in[:], gg[0:2, 0:1]).then_inc(
                        cc_sem, 16)
                    nc.gpsimd.collective_compute(
                        "AllGather", OP.bypass,
                        replica_groups=[list(range(NCORES))],
                        ins=[cc_in[:]], outs=[cc_out[:]],
                    )._wait_ge(cc_sem, 16).then_inc(cc_sem, 1)
                    nc.gpsimd.dma_start(ccsb[:], cc_out[None, :])._wait_ge(
                        cc_sem, 17).then_inc(cc_sem, 16)
                    nc.gpsimd.engine_nop()._wait_ge(cc_sem, 33)

            sg = small.tile([1, 4], F32, tag="sg")
            nc.vector.tensor_reduce(sg[:, 0:1], ccsb[:, 0:2 * NCORES:2],
                                    op=OP.max, axis=mybir.AxisListType.X)
            nc.vector.tensor_reduce(sg[:, 1:2], ccsb[:, 1:2 * NCORES:2],
                                    op=OP.max, axis=mybir.AxisListType.X)
            # sigma = emax / (vmax + 1e-8)
            nc.vector.tensor_scalar(sg[:, 2:3], sg[:, 0:1], 1e-8, None,
                                    op0=OP.add)
            nc.vector.reciprocal_approx_fast(out=sg[:, 0:1], in_=sg[:, 2:3])
            nc.vector.tensor_tensor(sg[:, 3:4], sg[:, 0:1], sg[:, 1:2],
                                    op=OP.mult)
            # broadcast sigma to all partitions via PE (ones-row matmul)
            sgp = ps2.tile([P, 8], F32, tag="ri")
            pe_touch(sg)
            nc.tensor.matmul(sgp[:, 0:1], onesf[0:1, 0:P], sg[0:1, 3:4],
                             start=True, stop=True)
            sgb = small.tile([P, 1], F32, tag="sgb")
            nc.vector.tensor_copy(sgb[:], sgp[:, 0:1])

            oq = [nc.sync, nc.scalar, nc.sync, nc.scalar, nc.sync]
            pieces = [(0, 0, HH), (0, HH, CH), (1, 0, HH),
                      (1, HH, HH + HH // 2), (1, HH + HH // 2, CH)]
            for i, (s, lo, hi) in enumerate(pieces):
                oview = out[s].rearrange("(p n) -> p n", p=P)
                qs = slice(lo, hi)
                nc.vector.tensor_scalar(vout[s][:, qs], vout[s][:, qs],
                                        sgb[:, 0:1], None, op0=OP.mult)
                oq[i].dma_start(oview[:, qs], vout[s][:, qs])
    nc.finalize()
    return nc


def _host_prep(inputs):
    gru = np.ascontiguousarray(np.asarray(inputs["gru_output"], np.float32))
    enh = np.ascontiguousarray(np.asarray(inputs["enhanced"], np.float32))
    noisy = np.ascontiguousarray(np.asarray(inputs["noisy"], np.float32))
    B = gru.shape[0]
    wts = _prep_weights(inputs["W1"], inputs["b1"], inputs["a1"],
                        inputs["W2"], inputs["b2"], inputs["a2"],
                        inputs["W3"], inputs["b3"])
    # bf16 weight pack
    wpk = np.zeros((128, BWCOLS), NPBF16)
    for n in ("w1t0", "w1t1", "a2xt0", "a2xt1", "b2xt"):
        off, r, c = BW[n]
        wpk[0:r, off:off + c] = wts[n].astype(NPBF16)
    off, r, c = BW["ident"]
    wpk[0:r, off:off + c] = np.eye(128).astype(NPBF16)
    m3 = _interp_m3()
    off, r, c = BW["m3d"]
    m3d = np.zeros((36, HOP))
    m3d[0:3] = 0.1 * m3
    m3d[32:35] = 0.1 * m3
    wpk[0:r, off:off + c] = m3d.astype(NPBF16)
    # f32 pack: r3 | bias1 | bias2 | spbias | eye(125)
    fpk = np.zeros((128, FPCOLS), np.float32)
    fpk[0:65, 0] = wts["r3"]
    fpk[0:128, 1] = wts["bias1"]
    fpk[0:65, 2] = wts["bias2"]
    fpk[0:P, 3] = wts["spb"]
    fpk[0:P, 4:4 + P] = np.eye(P, dtype=np.float32)

    noib = noisy.astype(NPBF16)
    enhb = enh.astype(NPBF16)

    per = B // NCORES
    in_maps = []
    for cr in range(NCORES):
        g = gru[cr * per:(cr + 1) * per].reshape(TB, GRU_H)
        gT = np.ascontiguousarray(g.T.astype(NPBF16))      # [256, 8000]
        xcat = np.empty((128, NCHK * 2 * CHK), NPBF16)
        for ch in range(NCHK):
            xcat[:, ch * 2 * CHK:ch * 2 * CHK + CHK] = \
                gT[0:128, ch * CHK:(ch + 1) * CHK]
            xcat[:, ch * 2 * CHK + CHK:(ch + 1) * 2 * CHK] = \
                gT[128:256, ch * CHK:(ch + 1) * CHK]
        m = {
            "xcat": xcat,
            "enh": np.ascontiguousarray(enhb[cr * per:(cr + 1) * per]),
            "noi": np.ascontiguousarray(noib[cr * per:(cr + 1) * per]),
            "wpk": wpk, "fpk": fpk,
        }
        in_maps.append(m)
    return in_maps


def kernel(trace=False, **inputs):
    in_maps = _host_prep(inputs)
    if "nc" not in _compiled:
        _compiled["nc"] = _build_nc()
    nc = _compiled["nc"]
    res = run_bass_kernel_spmd(nc, in_maps, list(range(NCORES)), trace=trace)
    outs = [np.asarray(res.results[c]["out"]).astype(np.float32)
            for c in range(NCORES)]
    full = np.concatenate(outs, axis=0)
    if trace:
        return full, res
    return full


if __name__ == "__main__":
    pass
